# revision 32
# baseline (speedup 1.0000x reference)
"""Causal self-attention (B=4, T=2048, C=1024, 16 heads, rope) on 8 trn2
NeuronCores, tensor-parallel over heads (2 heads/core).

All-bf16 datapath (f32 PSUM accumulation). Rope's rotate-half runs as a
swap-matrix matmul on the PE (a partition-shuffle-DMA variant was faster on
paper but nondeterministic on hardware). Work is software-pipelined at
(chunk, head)-unit granularity: each unit's softmax-denominator broadcast
(ones-stationary matmul), normalization, and the chunk's output-projection
matmuls are deferred and sprinkled between the score/PV iterations of the
NEXT unit, so the Ln/Exp reciprocal chain never head-blocks the tensor
queue. x slabs are prefetched one unit ahead through a dedicated sync-queue
slot; per-chunk x / weight / output tiles move as single 3D-AP DMAs.

Each core gets the full token stream plus its head-group's W_attn columns /
W_proj rows, computes a full-shape partial of the output projection in bf16,
and the host sums the 8 partials (the all-reduce) in f32 and transposes back.
"""

import ml_dtypes
import numpy as np

import concourse.bacc as bacc
import concourse.mybir as mybir
import concourse.tile as tile
from concourse.bass_utils import run_bass_kernel_spmd

F32 = mybir.dt.float32
BF16 = mybir.dt.bfloat16
AF = mybir.ActivationFunctionType

B, T, C = 4, 2048, 1024
N_HEAD, HEAD_DIM = 16, 64
N_CORES = 8
HPC = N_HEAD // N_CORES          # heads per core = 2
HF = HPC * HEAD_DIM              # per-core head features = 128
NT = B * T                       # 8192 tokens
KT = C // 128                    # 8 contraction tiles for qkv proj
QC = 512                         # query-chunk width
NQC = T // QC                    # 4 query chunks per batch
NKB = T // 128                   # key blocks per batch
ROPE_BASE = 10000.0
SCALE = 1.0 / 8.0                # 1/sqrt(HEAD_DIM)

_PROGRAM = None


def _patch_act_tables():
    """Make Exp/Ln resolve only to the combined natural_log_exp set so the
    table-load pass doesn't thrash between exp-only and ln-only sets."""
    import concourse.bacc as _bacc_mod
    from concourse import hw_specs as _hw

    if getattr(_bacc_mod, "_act_tables_patched", False):
        return
    _orig = _hw.get_activation_tables

    def _patched(arch):
        tabs = {k: set(v) for k, v in _orig(arch).items()}
        if "natural_log_exp_and_others" in tabs:
            for name, fns in tabs.items():
                if name != "natural_log_exp_and_others":
                    fns.discard(AF.Exp)
                    fns.discard(AF.Ln)
        return tabs

    _bacc_mod.get_activation_tables = _patched
    _bacc_mod._act_tables_patched = True


def _build_program():
    _patch_act_tables()
    nc = bacc.Bacc(None, target_bir_lowering=False)

    xT = nc.dram_tensor("xT", [C, NT], BF16, kind="ExternalInput")
    wq = nc.dram_tensor("wq", [C, HF], BF16, kind="ExternalInput")
    wk = nc.dram_tensor("wk", [C, HF], BF16, kind="ExternalInput")
    wv = nc.dram_tensor("wv", [C, HF], BF16, kind="ExternalInput")
    wp = nc.dram_tensor("wp", [HF, C], BF16, kind="ExternalInput")
    identd = nc.dram_tensor("identd", [128, 128], BF16, kind="ExternalInput")
    swapd = nc.dram_tensor("swapd", [128, 128], BF16, kind="ExternalInput")
    onesdd = nc.dram_tensor("onesdd", [128, 64], BF16, kind="ExternalInput")
    cosd = nc.dram_tensor("cosd", [HF, T], BF16, kind="ExternalInput")
    ssind = nc.dram_tensor("ssind", [HF, T], BF16, kind="ExternalInput")
    outT = nc.dram_tensor("outT", [C, NT], BF16, kind="ExternalOutput")

    xTr = xT[:].rearrange("(kt p) n -> p kt n", p=128)
    outTr = outT[:].rearrange("(of p) n -> p of n", p=128)

    with tile.TileContext(nc) as tc:
        with (
            tc.tile_pool(name="const", bufs=1) as cpool,
            tc.tile_pool(name="sx", bufs=4) as sx,
            tc.tile_pool(name="srope", bufs=2) as srope,
            tc.tile_pool(name="sv", bufs=2) as sv,
            tc.tile_pool(name="schunk", bufs=3) as schunk,
            tc.tile_pool(name="spt", bufs=6) as spt,
            tc.tile_pool(name="snorm", bufs=3) as snorm,
            tc.tile_pool(name="syn", bufs=2) as syn,
            tc.tile_pool(name="sstg", bufs=3) as sstg,
            tc.tile_pool(name="pq", bufs=3, space="PSUM") as pq,
            tc.tile_pool(name="psc", bufs=3, space="PSUM") as psc,
            tc.tile_pool(name="py", bufs=2, space="PSUM") as py,
        ):
            # ---- constants; x-slab prefetches for the first two chunks are
            # interleaved right after the first weight slab so the PE can
            # start within ~3us of kernel start ----
            wqs = cpool.tile([128, KT, 128], BF16, tag="wqs")
            nc.sync.dma_start(wqs[:], wq[:].rearrange("(kt p) m -> p kt m", p=128))
            wks = cpool.tile([128, KT, 128], BF16, tag="wks")
            wvs = cpool.tile([128, KT, 128], BF16, tag="wvs")
            cost = cpool.tile([128, T], BF16, tag="cost")
            ssint = cpool.tile([128, T], BF16, tag="ssint")
            ident = cpool.tile([128, 128], BF16, tag="ident")
            swap = cpool.tile([128, 128], BF16, tag="swap")
            onesd = cpool.tile([128, 64], BF16, tag="onesd")
            wps = cpool.tile([128, KT, 128], BF16, tag="wps")

            def load_consts():
                nc.sync.dma_start(cost[:], cosd[:])
                nc.sync.dma_start(ssint[:], ssind[:])
                nc.sync.dma_start(wvs[:], wv[:].rearrange("(kt p) m -> p kt m", p=128))
                nc.sync.dma_start(ident[:], identd[:])
                nc.sync.dma_start(swap[:], swapd[:])
                nc.sync.dma_start(onesd[:], onesdd[:])
                nc.sync.dma_start(wps[:], wp[:].rearrange("p (of m) -> p of m", m=128))

            state = {}    # per-batch rope_q / rope_k / vv
            ynorms = {}   # per-chunk normalized attention output
            xtiles = {}   # per-chunk prefetched x slabs

            def prefetch_x(j):
                """Issue the x-slab DMA for chunk j (no deps; heads the sync q)."""
                b, c = divmod(j, NQC)
                boff = b * T
                xs = sx.tile([128, KT, QC], BF16, tag="xs", name=f"xs{j}")
                nc.sync.dma_start(
                    xs[:], xTr[:, :, boff + c * QC: boff + (c + 1) * QC])
                xtiles[j] = xs

            def frontend(j):
                """QKV + rope + v^T for 512-token chunk j (= batch b, chunk c)."""
                b, c = divmod(j, NQC)
                if c == 0:
                    state[b] = {
                        "rope_q": srope.tile([128, T], BF16, tag="rope_q", name=f"ropeq{b}"),
                        "rope_k": srope.tile([128, T], BF16, tag="rope_k", name=f"ropek{b}"),
                        "vv": sv.tile([128, NKB * 130], BF16, tag="vv", name=f"vv{b}"),
                    }
                    nc.sync.dma_start(
                        state[b]["vv"][:, 64:NKB * 130:65], onesd[:, 0:32])
                st = state[b]
                cc = slice(c * QC, (c + 1) * QC)
                xs = xtiles.pop(j)
                for wslab, kind in ((wqs, "q"), (wks, "k")):
                    ps = pq.tile([128, QC], F32, tag="mm", name=f"ps{j}{kind}")
                    for kt in range(KT):
                        nc.tensor.matmul(
                            ps[:], wslab[:, kt, :], xs[:, kt, :],
                            start=(kt == 0), stop=(kt == KT - 1),
                        )
                    raw = schunk.tile([128, QC], BF16, tag="rawc", name=f"raw{j}{kind}")
                    nc.vector.tensor_copy(raw[:], ps[:])
                    ta = schunk.tile([128, QC], BF16, tag="tac", name=f"ta{j}{kind}")
                    nc.vector.tensor_mul(ta[:], ps[:], cost[:, cc])
                    dst = st["rope_q"] if kind == "q" else st["rope_k"]
                    rot = pq.tile([128, QC], F32, tag="mm", name=f"rot{j}{kind}")
                    nc.tensor.matmul(rot[:], swap[:], raw[:], start=True, stop=True)
                    tb = schunk.tile([128, QC], BF16, tag="tbc", name=f"tb{j}{kind}")
                    nc.vector.tensor_mul(tb[:], rot[:], ssint[:, cc])
                    nc.gpsimd.tensor_add(dst[:, cc], ta[:], tb[:])
                # v: weight-stationary projection then PE-transpose to [tok, dim]
                vv = st["vv"]
                psv = pq.tile([128, QC], F32, tag="mm", name=f"psv{j}")
                for kt in range(KT):
                    nc.tensor.matmul(
                        psv[:], wvs[:, kt, :], xs[:, kt, :],
                        start=(kt == 0), stop=(kt == KT - 1),
                    )
                vtc = schunk.tile([128, QC], BF16, tag="vtc", name=f"vt{j}")
                nc.vector.tensor_copy(vtc[:], psv[:])
                pvt = pq.tile([128, QC], BF16, tag="mm", name=f"pvt{j}")
                for t4 in range(4):
                    nc.tensor.transpose(
                        pvt[:, t4 * 128:(t4 + 1) * 128],
                        vtc[:, t4 * 128:(t4 + 1) * 128], ident[:])
                    ki = 4 * c + t4
                    nc.vector.tensor_copy(
                        vv[:, ki * 130: ki * 130 + 64], pvt[:, t4 * 128: t4 * 128 + 64])
                    nc.vector.tensor_copy(
                        vv[:, ki * 130 + 65: ki * 130 + 129], pvt[:, t4 * 128 + 64: t4 * 128 + 128])

            def kiloop(b, qc, h, pending):
                """Scores/exp/PV for one (chunk, head) unit; lag-2 ki pipeline.
                `pending` = deferred tensor-op closures (prev chunk's out-proj)."""
                st = state[b]
                rope_q, rope_k, vv = st["rope_q"], st["rope_k"], st["vv"]
                hb = h * 64
                qs = qc * QC
                nki = (qs + QC) // 128
                y = py.tile([65, QC], F32, tag="y", name=f"y{b}_{qc}_{h}")
                pts = {}

                def emit_score(ki):
                    c0 = max(0, ki * 128 - qs)
                    sc = psc.tile([128, QC], F32, tag="sc", name=f"sc{b}_{qc}_{h}_{ki}")
                    nc.tensor.matmul(
                        sc[:, c0:QC],
                        rope_k[hb:hb + 64, ki * 128:(ki + 1) * 128],
                        rope_q[hb:hb + 64, qs + c0:qs + QC],
                        start=True, stop=True,
                    )
                    pt = spt.tile([128, QC], BF16, tag="pt", name=f"pt{b}_{qc}_{h}_{ki}")
                    nc.scalar.activation(pt[:, c0:QC], sc[:, c0:QC], AF.Exp, scale=SCALE)
                    if ki * 128 >= qs:  # diagonal block: zero where k > q
                        nc.gpsimd.affine_select(
                            pt[:, c0:c0 + 128], pt[:, c0:c0 + 128],
                            pattern=[[1, 128]],
                            compare_op=mybir.AluOpType.is_ge,
                            fill=0.0, base=0, channel_multiplier=-1,
                        )
                    pts[ki] = (pt, c0)

                def emit_pv(ki):
                    pt, c0 = pts.pop(ki)
                    nc.tensor.matmul(
                        y[0:65, c0:QC],
                        vv[:, ki * 130 + 65 * h: ki * 130 + 65 * h + 65],
                        pt[:, c0:QC],
                        start=(ki == 0), stop=(ki == nki - 1),
                    )

                for ki in range(nki):
                    emit_score(ki)
                    if ki >= 3:
                        emit_pv(ki - 3)
                        for _ in range(2):
                            if pending:
                                pending.pop(0)()
                emit_pv(nki - 3)
                emit_pv(nki - 2)
                emit_pv(nki - 1)
                while pending:
                    pending.pop(0)()
                return y

            def norm_scalar(b, qc, h, y):
                """Softmax denominator -> log -> negated exp (reciprocal)."""
                lnt = snorm.tile([65, QC], F32, tag="lnt", name=f"ln{b}_{qc}_{h}")
                nc.scalar.activation(lnt[64:65, :], y[64:65, :], AF.Ln)
                rec = snorm.tile([65, QC], BF16, tag="rec", name=f"rc{b}_{qc}_{h}")
                nc.scalar.activation(rec[64:65, :], lnt[64:65, :], AF.Exp, scale=-1.0)
                return rec

            def norm_deferred(b, qc, h, y, rec):
                """Broadcast reciprocal across partitions (matmul) + normalize.
                Returned closures run inside the NEXT unit's kiloop."""
                def c_bc():
                    bc = pq.tile([128, QC], F32, tag="mm", name=f"bb{b}_{qc}_{h}")
                    nc.tensor.matmul(
                        bc[0:64, :], onesd[64:65, 0:64], rec[64:65, :],
                        start=True, stop=True,
                    )
                    bcs = snorm.tile([64, QC], BF16, tag="bcs", name=f"bc{b}_{qc}_{h}")
                    nc.vector.tensor_copy(bcs[:], bc[0:64, :])
                    if h == 0:
                        yn = syn.tile([128, QC], BF16, tag="ynorm", name=f"yn{b}_{qc}")
                        ynorms[(b, qc)] = yn
                        nc.vector.tensor_mul(yn[0:64, :], y[0:64, :], bcs[:])
                    else:
                        yn = ynorms[(b, qc)]
                        hn = snorm.tile([64, QC], BF16, tag="hn", name=f"hn{b}_{qc}")
                        nc.vector.tensor_mul(hn[:], y[0:64, :], bcs[:])
                        nc.sync.dma_start(yn[64:128, :], hn[:])
                return [c_bc]

            def make_ops(b, qc):
                """Deferred out-projection closures for chunk (b, qc)."""
                yn = ynorms.pop((b, qc))
                boff = b * T
                qs = qc * QC
                stg = sstg.tile([128, KT, QC], BF16, tag="stg", name=f"stg{b}_{qc}")
                ops = []

                def op(of):
                    def emit():
                        o = pq.tile([128, QC], F32, tag="mm", name=f"op{b}_{qc}_{of}")
                        nc.tensor.matmul(o[:], wps[:, of, :], yn[:], start=True, stop=True)
                        nc.vector.tensor_copy(stg[:, of, :], o[:])
                    return emit

                for of in range(KT):
                    ops.append(op(of))

                def store():
                    nc.sync.dma_start(
                        outTr[:, :, boff + qs: boff + qs + QC], stg[:])
                ops.append(store)
                return ops

            # ---- software pipeline over (batch, chunk, head) units ----
            seq = [(b, c) for b in range(B) for c in range(NQC)]
            units = [(b, c, h) for (b, c) in seq for h in range(HPC)]
            prefetch_x(0)
            nc.sync.dma_start(wks[:], wk[:].rearrange("(kt p) m -> p kt m", p=128))
            prefetch_x(1)
            load_consts()
            frontend(0)
            frontend(1)
            pending = []       # closures sprinkled into the next kiloop
            opschunk = None    # chunk whose out-proj is ready to defer
            for u, (b, qc, h) in enumerate(units):
                j = u // 2
                if h == 0 and j + 2 < len(seq):
                    prefetch_x(j + 2)
                y = kiloop(b, qc, h, pending)
                rec = norm_scalar(b, qc, h, y)
                pending = norm_deferred(b, qc, h, y, rec)
                if opschunk is not None:
                    pending += make_ops(*opschunk)
                    opschunk = None
                if h == 1:
                    opschunk = (b, qc)
                    if j + 2 < len(seq):
                        frontend(j + 2)
                    if qc == NQC - 1:
                        state.pop(b)
            for fn in pending:
                fn()
            for fn in make_ops(*opschunk):
                fn()
    nc.finalize()
    return nc


def _program():
    global _PROGRAM
    if _PROGRAM is None:
        _PROGRAM = _build_program()
    return _PROGRAM


def _rope_tables():
    inv_freq = 1.0 / (ROPE_BASE ** (np.arange(0, HEAD_DIM, 2, dtype=np.float32) / HEAD_DIM))
    t = np.arange(T, dtype=np.float32)
    freqs = np.outer(t, inv_freq).astype(np.float32)        # [T, 32]
    emb = np.concatenate([freqs, freqs], axis=1)            # [T, 64]
    cos = np.cos(emb).astype(np.float32)                    # [T, 64]
    sin = np.sin(emb).astype(np.float32)
    ssin = sin.copy()
    ssin[:, :32] *= -1.0                                    # signed for rotate_half
    cosT = np.ascontiguousarray(cos.T)                      # [64, T]
    ssinT = np.ascontiguousarray(ssin.T)
    cos2 = np.concatenate([cosT] * HPC, axis=0)             # [128, T]
    ssin2 = np.concatenate([ssinT] * HPC, axis=0)
    return cos2, ssin2


def _swap_matrix():
    s = np.zeros((128, 128), dtype=np.float32)
    for hb in (0, 64):
        for m in range(32):
            s[hb + 32 + m, hb + m] = 1.0      # rot[m] = raw[m+32]
            s[hb + m, hb + 32 + m] = 1.0      # rot[m+32] = raw[m]
    return s


def _prep_in_maps(x, W_attn, W_proj):
    bf16 = ml_dtypes.bfloat16
    x = np.asarray(x, dtype=np.float32)
    W_attn = np.asarray(W_attn, dtype=np.float32)
    W_proj = np.asarray(W_proj, dtype=np.float32)
    xT = np.ascontiguousarray(x.reshape(NT, C).T).astype(bf16)
    cos2, ssin2 = _rope_tables()
    cos2 = cos2.astype(bf16)
    ssin2 = ssin2.astype(bf16)
    ident = np.eye(128, dtype=np.float32).astype(bf16)
    swap = _swap_matrix().astype(bf16)
    ones = np.ones((128, 64), dtype=np.float32).astype(bf16)
    in_maps = []
    for i in range(N_CORES):
        cs = i * HF
        in_maps.append({
            "xT": xT,
            "wq": np.ascontiguousarray(W_attn[:, cs:cs + HF]).astype(bf16),
            "wk": np.ascontiguousarray(W_attn[:, C + cs:C + cs + HF]).astype(bf16),
            "wv": np.ascontiguousarray(W_attn[:, 2 * C + cs:2 * C + cs + HF]).astype(bf16),
            "wp": np.ascontiguousarray(W_proj[cs:cs + HF, :]).astype(bf16),
            "identd": ident,
            "swapd": swap,
            "onesdd": ones,
            "cosd": cos2,
            "ssind": ssin2,
        })
    return in_maps


def _run(in_maps, trace=False, **kwargs):
    return run_bass_kernel_spmd(
        _program(), in_maps, core_ids=list(range(N_CORES)), trace=trace, **kwargs
    )


def kernel(x, W_attn, W_proj):
    in_maps = _prep_in_maps(x, W_attn, W_proj)
    res = _run(in_maps)
    acc = np.zeros((C, NT), dtype=np.float32)
    for r in res.results:
        acc += r["outT"].astype(np.float32)
    return np.ascontiguousarray(acc.T).reshape(B, T, C)


# revision 35
# speedup vs baseline: 1.1009x; 1.1009x over previous
"""Causal self-attention (B=4, T=2048, C=1024, 16 heads, rope) on 8 trn2
NeuronCores, tensor-parallel over heads (2 heads/core).

All-bf16 datapath (f32 PSUM accumulation). Rope's rotate-half runs as a
swap-matrix matmul on the PE (a partition-shuffle-DMA variant was faster on
paper but nondeterministic on hardware). Work is software-pipelined at
(chunk, head)-unit granularity: each unit's softmax-denominator broadcast
(ones-stationary matmul), normalization, and the chunk's output-projection
matmuls are deferred and sprinkled between the score/PV iterations of the
NEXT unit, so the Ln/Exp reciprocal chain never head-blocks the tensor
queue. x slabs are prefetched one unit ahead through a dedicated sync-queue
slot; per-chunk x / weight / output tiles move as single 3D-AP DMAs.

Each core gets the full token stream plus its head-group's W_attn columns /
W_proj rows, computes a full-shape partial of the output projection in bf16,
and the host sums the 8 partials (the all-reduce) in f32 and transposes back.
"""

import ml_dtypes
import numpy as np

import concourse.bacc as bacc
import concourse.mybir as mybir
import concourse.tile as tile
from concourse.bass_utils import run_bass_kernel_spmd

F32 = mybir.dt.float32
BF16 = mybir.dt.bfloat16
AF = mybir.ActivationFunctionType

B, T, C = 4, 2048, 1024
N_HEAD, HEAD_DIM = 16, 64
N_CORES = 8
HPC = N_HEAD // N_CORES          # heads per core = 2
HF = HPC * HEAD_DIM              # per-core head features = 128
NT = B * T                       # 8192 tokens
KT = C // 128                    # 8 contraction tiles for qkv proj
QC = 512                         # query-chunk width
NQC = T // QC                    # 4 query chunks per batch
NKB = T // 128                   # key blocks per batch
ROPE_BASE = 10000.0
SCALE = 1.0 / 8.0                # 1/sqrt(HEAD_DIM)

_PROGRAM = None


def _patch_act_tables():
    """Make Exp/Ln resolve only to the combined natural_log_exp set so the
    table-load pass doesn't thrash between exp-only and ln-only sets."""
    import concourse.bacc as _bacc_mod
    from concourse import hw_specs as _hw

    if getattr(_bacc_mod, "_act_tables_patched", False):
        return
    _orig = _hw.get_activation_tables

    def _patched(arch):
        tabs = {k: set(v) for k, v in _orig(arch).items()}
        if "natural_log_exp_and_others" in tabs:
            for name, fns in tabs.items():
                if name != "natural_log_exp_and_others":
                    fns.discard(AF.Exp)
                    fns.discard(AF.Ln)
        return tabs

    _bacc_mod.get_activation_tables = _patched
    _bacc_mod._act_tables_patched = True


def _build_program():
    _patch_act_tables()
    nc = bacc.Bacc(None, target_bir_lowering=False)

    xT = nc.dram_tensor("xT", [C, NT], BF16, kind="ExternalInput")
    wq = nc.dram_tensor("wq", [C, HF], BF16, kind="ExternalInput")
    wk = nc.dram_tensor("wk", [C, HF], BF16, kind="ExternalInput")
    wv = nc.dram_tensor("wv", [C, HF], BF16, kind="ExternalInput")
    wp = nc.dram_tensor("wp", [HF, C], BF16, kind="ExternalInput")
    identd = nc.dram_tensor("identd", [128, 128], BF16, kind="ExternalInput")
    swapd = nc.dram_tensor("swapd", [128, 128], BF16, kind="ExternalInput")
    onesdd = nc.dram_tensor("onesdd", [128, 64], BF16, kind="ExternalInput")
    cosd = nc.dram_tensor("cosd", [HF, T], BF16, kind="ExternalInput")
    ssind = nc.dram_tensor("ssind", [HF, T], BF16, kind="ExternalInput")
    outT = nc.dram_tensor("outT", [C, NT], BF16, kind="ExternalOutput")

    xTr = xT[:].rearrange("(kt p) n -> p kt n", p=128)
    outTr = outT[:].rearrange("(of p) n -> p of n", p=128)

    with tile.TileContext(nc) as tc:
        with (
            tc.tile_pool(name="const", bufs=1) as cpool,
            tc.tile_pool(name="sx", bufs=5) as sx,
            tc.tile_pool(name="srope", bufs=2) as srope,
            tc.tile_pool(name="sv", bufs=2) as sv,
            tc.tile_pool(name="schunk", bufs=5) as schunk,
            tc.tile_pool(name="spt", bufs=8) as spt,
            tc.tile_pool(name="snorm", bufs=4) as snorm,
            tc.tile_pool(name="syn", bufs=3) as syn,
            tc.tile_pool(name="sstg", bufs=4) as sstg,
            tc.tile_pool(name="pq", bufs=3, space="PSUM") as pq,
            tc.tile_pool(name="psc", bufs=3, space="PSUM") as psc,
            tc.tile_pool(name="py", bufs=2, space="PSUM") as py,
        ):
            # ---- constants; x-slab prefetches for the first two chunks are
            # interleaved right after the first weight slab so the PE can
            # start within ~3us of kernel start ----
            wqs = cpool.tile([128, KT, 128], BF16, tag="wqs")
            nc.sync.dma_start(wqs[:], wq[:].rearrange("(kt p) m -> p kt m", p=128))
            wks = cpool.tile([128, KT, 128], BF16, tag="wks")
            wvs = cpool.tile([128, KT, 128], BF16, tag="wvs")
            cost = cpool.tile([128, T], BF16, tag="cost")
            ssint = cpool.tile([128, T], BF16, tag="ssint")
            ident = cpool.tile([128, 128], BF16, tag="ident")
            swap = cpool.tile([128, 128], BF16, tag="swap")
            onesd = cpool.tile([128, 64], BF16, tag="onesd")
            wps = cpool.tile([128, KT, 128], BF16, tag="wps")

            def load_consts():
                nc.sync.dma_start(cost[:], cosd[:])
                nc.sync.dma_start(ssint[:], ssind[:])
                nc.sync.dma_start(wvs[:], wv[:].rearrange("(kt p) m -> p kt m", p=128))
                nc.sync.dma_start(ident[:], identd[:])
                nc.sync.dma_start(swap[:], swapd[:])
                nc.sync.dma_start(onesd[:], onesdd[:])
                nc.sync.dma_start(wps[:], wp[:].rearrange("p (of m) -> p of m", m=128))

            state = {}    # per-batch rope_q / rope_k / vv
            ynorms = {}   # per-chunk normalized attention output
            xtiles = {}   # per-chunk prefetched x slabs

            def prefetch_x(j):
                """Issue the x-slab DMA for chunk j (no deps; heads the sync q)."""
                b, c = divmod(j, NQC)
                boff = b * T
                xs = sx.tile([128, KT, QC], BF16, tag="xs", name=f"xs{j}")
                nc.sync.dma_start(
                    xs[:], xTr[:, :, boff + c * QC: boff + (c + 1) * QC])
                xtiles[j] = xs

            def frontend(j):
                """QKV + rope + v^T for 512-token chunk j (= batch b, chunk c)."""
                b, c = divmod(j, NQC)
                if c == 0:
                    state[b] = {
                        "rope_q": srope.tile([128, T], BF16, tag="rope_q", name=f"ropeq{b}"),
                        "rope_k": srope.tile([128, T], BF16, tag="rope_k", name=f"ropek{b}"),
                        "vv": sv.tile([128, NKB * 130], BF16, tag="vv", name=f"vv{b}"),
                    }
                    nc.sync.dma_start(
                        state[b]["vv"][:, 64:NKB * 130:65], onesd[:, 0:32])
                st = state[b]
                cc = slice(c * QC, (c + 1) * QC)
                xs = xtiles.pop(j)
                for wslab, kind in ((wqs, "q"), (wks, "k")):
                    ps = pq.tile([128, QC], F32, tag="mm", name=f"ps{j}{kind}")
                    for kt in range(KT):
                        nc.tensor.matmul(
                            ps[:], wslab[:, kt, :], xs[:, kt, :],
                            start=(kt == 0), stop=(kt == KT - 1),
                        )
                    raw = schunk.tile([128, QC], BF16, tag="rawc", name=f"raw{j}{kind}")
                    nc.vector.tensor_copy(raw[:], ps[:])
                    ta = schunk.tile([128, QC], BF16, tag="tac", name=f"ta{j}{kind}")
                    nc.vector.tensor_mul(ta[:], ps[:], cost[:, cc])
                    dst = st["rope_q"] if kind == "q" else st["rope_k"]
                    rot = pq.tile([128, QC], F32, tag="mm", name=f"rot{j}{kind}")
                    nc.tensor.matmul(rot[:], swap[:], raw[:], start=True, stop=True)
                    tb = schunk.tile([128, QC], BF16, tag="tbc", name=f"tb{j}{kind}")
                    nc.vector.tensor_mul(tb[:], rot[:], ssint[:, cc])
                    nc.gpsimd.tensor_add(dst[:, cc], ta[:], tb[:])
                # v: weight-stationary projection then PE-transpose to [tok, dim]
                vv = st["vv"]
                psv = pq.tile([128, QC], F32, tag="mm", name=f"psv{j}")
                for kt in range(KT):
                    nc.tensor.matmul(
                        psv[:], wvs[:, kt, :], xs[:, kt, :],
                        start=(kt == 0), stop=(kt == KT - 1),
                    )
                vtc = schunk.tile([128, QC], BF16, tag="vtc", name=f"vt{j}")
                nc.vector.tensor_copy(vtc[:], psv[:])
                pvt = pq.tile([128, QC], BF16, tag="mm", name=f"pvt{j}")
                for t4 in range(4):
                    nc.tensor.transpose(
                        pvt[:, t4 * 128:(t4 + 1) * 128],
                        vtc[:, t4 * 128:(t4 + 1) * 128], ident[:])
                    ki = 4 * c + t4
                    nc.vector.tensor_copy(
                        vv[:, ki * 130: ki * 130 + 64], pvt[:, t4 * 128: t4 * 128 + 64])
                    nc.vector.tensor_copy(
                        vv[:, ki * 130 + 65: ki * 130 + 129], pvt[:, t4 * 128 + 64: t4 * 128 + 128])

            def kiloop(b, qc, h, pending):
                """Scores/exp/PV for one (chunk, head) unit; lag-2 ki pipeline.
                `pending` = deferred tensor-op closures (prev chunk's out-proj)."""
                st = state[b]
                rope_q, rope_k, vv = st["rope_q"], st["rope_k"], st["vv"]
                hb = h * 64
                qs = qc * QC
                nki = (qs + QC) // 128
                y = py.tile([65, QC], F32, tag="y", name=f"y{b}_{qc}_{h}")
                pts = {}

                def emit_score(ki):
                    c0 = max(0, ki * 128 - qs)
                    sc = psc.tile([128, QC], F32, tag="sc", name=f"sc{b}_{qc}_{h}_{ki}")
                    nc.tensor.matmul(
                        sc[:, c0:QC],
                        rope_k[hb:hb + 64, ki * 128:(ki + 1) * 128],
                        rope_q[hb:hb + 64, qs + c0:qs + QC],
                        start=True, stop=True,
                    )
                    pt = spt.tile([128, QC], BF16, tag="pt", name=f"pt{b}_{qc}_{h}_{ki}")
                    nc.scalar.activation(pt[:, c0:QC], sc[:, c0:QC], AF.Exp, scale=SCALE)
                    if ki * 128 >= qs:  # diagonal block: zero where k > q
                        nc.gpsimd.affine_select(
                            pt[:, c0:c0 + 128], pt[:, c0:c0 + 128],
                            pattern=[[1, 128]],
                            compare_op=mybir.AluOpType.is_ge,
                            fill=0.0, base=0, channel_multiplier=-1,
                        )
                    pts[ki] = (pt, c0)

                def emit_pv(ki):
                    pt, c0 = pts.pop(ki)
                    nc.tensor.matmul(
                        y[0:65, c0:QC],
                        vv[:, ki * 130 + 65 * h: ki * 130 + 65 * h + 65],
                        pt[:, c0:QC],
                        start=(ki == 0), stop=(ki == nki - 1),
                    )

                for ki in range(nki):
                    emit_score(ki)
                    if ki >= 3:
                        emit_pv(ki - 3)
                        for _ in range(2):
                            if pending:
                                pending.pop(0)()
                emit_pv(nki - 3)
                emit_pv(nki - 2)
                emit_pv(nki - 1)
                while pending:
                    pending.pop(0)()
                return y

            def norm_scalar(b, qc, h, y):
                """Softmax denominator -> log -> negated exp (reciprocal)."""
                lnt = snorm.tile([65, QC], F32, tag="lnt", name=f"ln{b}_{qc}_{h}")
                nc.scalar.activation(lnt[64:65, :], y[64:65, :], AF.Ln)
                rec = snorm.tile([65, QC], BF16, tag="rec", name=f"rc{b}_{qc}_{h}")
                nc.scalar.activation(rec[64:65, :], lnt[64:65, :], AF.Exp, scale=-1.0)
                return rec

            def norm_deferred(b, qc, h, y, rec):
                """Broadcast reciprocal across partitions (matmul) + normalize.
                Returned closures run inside the NEXT unit's kiloop."""
                def c_bc():
                    bc = pq.tile([128, QC], F32, tag="mm", name=f"bb{b}_{qc}_{h}")
                    nc.tensor.matmul(
                        bc[0:64, :], onesd[64:65, 0:64], rec[64:65, :],
                        start=True, stop=True,
                    )
                    bcs = snorm.tile([64, QC], BF16, tag="bcs", name=f"bc{b}_{qc}_{h}")
                    nc.vector.tensor_copy(bcs[:], bc[0:64, :])
                    if h == 0:
                        yn = syn.tile([128, QC], BF16, tag="ynorm", name=f"yn{b}_{qc}")
                        ynorms[(b, qc)] = yn
                        nc.vector.tensor_mul(yn[0:64, :], y[0:64, :], bcs[:])
                    else:
                        yn = ynorms[(b, qc)]
                        hn = snorm.tile([64, QC], BF16, tag="hn", name=f"hn{b}_{qc}")
                        nc.vector.tensor_mul(hn[:], y[0:64, :], bcs[:])
                        nc.sync.dma_start(yn[64:128, :], hn[:])
                return [c_bc]

            def make_ops(b, qc):
                """Deferred out-projection closures for chunk (b, qc)."""
                yn = ynorms.pop((b, qc))
                boff = b * T
                qs = qc * QC
                stg = sstg.tile([128, KT, QC], BF16, tag="stg", name=f"stg{b}_{qc}")
                ops = []

                def op(of):
                    def emit():
                        o = pq.tile([128, QC], F32, tag="mm", name=f"op{b}_{qc}_{of}")
                        nc.tensor.matmul(o[:], wps[:, of, :], yn[:], start=True, stop=True)
                        nc.vector.tensor_copy(stg[:, of, :], o[:])
                    return emit

                for of in range(KT):
                    ops.append(op(of))

                def store():
                    nc.sync.dma_start(
                        outTr[:, :, boff + qs: boff + qs + QC], stg[:])
                ops.append(store)
                return ops

            # ---- software pipeline over (batch, chunk, head) units ----
            seq = [(b, c) for b in range(B) for c in range(NQC)]
            units = [(b, c, h) for (b, c) in seq for h in range(HPC)]
            prefetch_x(0)
            nc.sync.dma_start(wks[:], wk[:].rearrange("(kt p) m -> p kt m", p=128))
            prefetch_x(1)
            load_consts()
            frontend(0)
            frontend(1)
            pending = []       # closures sprinkled into the next kiloop
            opschunk = None    # chunk whose out-proj is ready to defer
            for u, (b, qc, h) in enumerate(units):
                j = u // 2
                if h == 0 and j + 2 < len(seq):
                    prefetch_x(j + 2)
                y = kiloop(b, qc, h, pending)
                rec = norm_scalar(b, qc, h, y)
                pending = norm_deferred(b, qc, h, y, rec)
                if opschunk is not None:
                    pending += make_ops(*opschunk)
                    opschunk = None
                if h == 1:
                    opschunk = (b, qc)
                    if j + 2 < len(seq):
                        frontend(j + 2)
                    if qc == NQC - 1:
                        state.pop(b)
            for fn in pending:
                fn()
            for fn in make_ops(*opschunk):
                fn()
    nc.finalize()
    return nc


def _program():
    global _PROGRAM
    if _PROGRAM is None:
        _PROGRAM = _build_program()
    return _PROGRAM


def _rope_tables():
    inv_freq = 1.0 / (ROPE_BASE ** (np.arange(0, HEAD_DIM, 2, dtype=np.float32) / HEAD_DIM))
    t = np.arange(T, dtype=np.float32)
    freqs = np.outer(t, inv_freq).astype(np.float32)        # [T, 32]
    emb = np.concatenate([freqs, freqs], axis=1)            # [T, 64]
    cos = np.cos(emb).astype(np.float32)                    # [T, 64]
    sin = np.sin(emb).astype(np.float32)
    ssin = sin.copy()
    ssin[:, :32] *= -1.0                                    # signed for rotate_half
    cosT = np.ascontiguousarray(cos.T)                      # [64, T]
    ssinT = np.ascontiguousarray(ssin.T)
    cos2 = np.concatenate([cosT] * HPC, axis=0)             # [128, T]
    ssin2 = np.concatenate([ssinT] * HPC, axis=0)
    return cos2, ssin2


def _swap_matrix():
    s = np.zeros((128, 128), dtype=np.float32)
    for hb in (0, 64):
        for m in range(32):
            s[hb + 32 + m, hb + m] = 1.0      # rot[m] = raw[m+32]
            s[hb + m, hb + 32 + m] = 1.0      # rot[m+32] = raw[m]
    return s


def _prep_in_maps(x, W_attn, W_proj):
    bf16 = ml_dtypes.bfloat16
    x = np.asarray(x, dtype=np.float32)
    W_attn = np.asarray(W_attn, dtype=np.float32)
    W_proj = np.asarray(W_proj, dtype=np.float32)
    xT = np.ascontiguousarray(x.reshape(NT, C).T).astype(bf16)
    cos2, ssin2 = _rope_tables()
    cos2 = cos2.astype(bf16)
    ssin2 = ssin2.astype(bf16)
    ident = np.eye(128, dtype=np.float32).astype(bf16)
    swap = _swap_matrix().astype(bf16)
    ones = np.ones((128, 64), dtype=np.float32).astype(bf16)
    in_maps = []
    for i in range(N_CORES):
        cs = i * HF
        in_maps.append({
            "xT": xT,
            "wq": np.ascontiguousarray(W_attn[:, cs:cs + HF]).astype(bf16),
            "wk": np.ascontiguousarray(W_attn[:, C + cs:C + cs + HF]).astype(bf16),
            "wv": np.ascontiguousarray(W_attn[:, 2 * C + cs:2 * C + cs + HF]).astype(bf16),
            "wp": np.ascontiguousarray(W_proj[cs:cs + HF, :]).astype(bf16),
            "identd": ident,
            "swapd": swap,
            "onesdd": ones,
            "cosd": cos2,
            "ssind": ssin2,
        })
    return in_maps


def _run(in_maps, trace=False, **kwargs):
    return run_bass_kernel_spmd(
        _program(), in_maps, core_ids=list(range(N_CORES)), trace=trace, **kwargs
    )


def kernel(x, W_attn, W_proj):
    in_maps = _prep_in_maps(x, W_attn, W_proj)
    res = _run(in_maps)
    acc = np.zeros((C, NT), dtype=np.float32)
    for r in res.results:
        acc += r["outT"].astype(np.float32)
    return np.ascontiguousarray(acc.T).reshape(B, T, C)
